# revision 2
# baseline (speedup 1.0000x reference)
"""GarNet v2: fp32r big-moving matmuls, diag-garbage psum, bf16 stores.

Math per example: w = exp(-d^2) [V=128,S=16]; out = (mean_V w)[s] * (w^T fi / V)[s,n].

Per core (bpc=512): host-transposed inputs fi_t [V,bpc,N], d_t [V,bpc,S].
8 blocks of 64 examples; d in pairs of blocks.
  w pipeline (per pair): square (DVE) -> exp (ACT) -> 16 wbar-col matmuls
    (w_stackT @ ones/V^2 -> [128,1]) -> wcol sbuf [128,16] (DVE copy)
  big matmuls (per block): 16 groups of 4 ex: lhsT=w[128,64] rhs=fi[128,256]
    fp32r (1cyc/row) -> psum [64,256] quarters; [128,2048] psum tile/block
  drain (DVE/ACT): ot_bf16[128,4,256] = psum * wcol-broadcast  (fold+convert)
  store ot (bf16, 0.5MB) -> out[b]; host extracts diagonal + converts.

Channels: SP/ACT direct queues + Pool SWDGE gathers (~1.7ns/KB serial on Pool).
"""

import numpy as np
from contextlib import ExitStack

import concourse.bass as bass
import concourse.tile as tile
from concourse import library_config, mybir

B, V, S, N = 4096, 128, 16, 64
NCORES = 8
BPC = B // NCORES          # 512
EB = 64                    # examples per block
NBLK = BPC // EB           # 8
NPAIR = NBLK // 2          # 4
ONES_VAL = 1.0 / (V * V)

f32 = mybir.dt.float32
f32r = mybir.dt.float32r
bf16 = mybir.dt.bfloat16
i16 = mybir.dt.int16


def build(name="garnet2", store_map="sssssggs", sq_act="0011",
          chain="d1,f2,f3,f4,d2,f5,d3,f6", warm_n=24,
          fi_direct="", drain_act="", obufs=6, fbufs=8, drain_mode="blocks", psplit=True, d_direct=""):
    nc = bass.Bass(name=name, dynamic_dma_scratch_size=16 * 1024)
    fi_t = nc.dram_tensor("fi_t", (V, BPC, N), f32, kind="ExternalInput")
    d_t = nc.dram_tensor("d_t", (V, BPC, S), f32, kind="ExternalInput")
    out = nc.dram_tensor("out", (NBLK, 128, 2048), bf16, kind="ExternalOutput")

    # gather row views: fi rows = 16 examples (4KB), d rows = 32 examples (2KB)
    fi_rows = fi_t.rearrange("v (j e) n -> (v j) (e n)", e=16)   # [4096, 1024]
    d_rows = d_t.rearrange("v (c e) s -> (v c) (e s)", e=32)     # [2048, 512]

    fi_sem = [nc.alloc_semaphore(f"fi_sem{i}") for i in range(NBLK)]
    d_sem = [nc.alloc_semaphore(f"d_sem{i}") for i in range(NPAIR)]

    with tile.TileContext(nc) as tc, ExitStack() as ctx:
        warm = ctx.enter_context(tc.tile_pool(name="warm", bufs=1))
        fipool = ctx.enter_context(tc.tile_pool(name="fipool", bufs=fbufs))
        dpool = ctx.enter_context(tc.tile_pool(name="dpool", bufs=4))
        opool = ctx.enter_context(tc.tile_pool(name="opool", bufs=obufs))
        wpool = ctx.enter_context(tc.tile_pool(name="wpool", bufs=2))
        psum = ctx.enter_context(tc.tile_pool(name="psum", bufs=4 if psplit else 2, space="PSUM"))

        # --- constants, indices ---
        with tc.high_priority():
            ones = warm.tile([128, 1], f32)
            nc.vector.memset(ones, ONES_VAL)
            wz = warm.tile([128, 512], f32)
            nc.vector.memset(wz, 0.0)
            wtmp = warm.tile([128, 1], f32)
            # preload the exp_and_others act table while head DMAs run
            nc.scalar.activation(wtmp, ones, mybir.ActivationFunctionType.Exp, scale=-1.0)
            fi_idx = {}
            iota_insts = []
            for b in range(2, NBLK):
                t = warm.tile([128, 32], i16, name=f"fi_idx{b}")
                nc.vector.memset(t, 0)
                # idx value for dst (p, j) = 32*p + 4*b + j
                it = nc.gpsimd.iota(
                    t[0:16].rearrange("p (hi lo) -> p hi lo", lo=8),
                    pattern=[[1, 4], [512, 8]],
                    base=4 * b,
                    channel_multiplier=32,
                )
                iota_insts.append(it)
                fi_idx[b] = t
            d_idx = {}
            for p in range(1, NPAIR):
                t = warm.tile([128, 32], i16, name=f"d_idx{p}")
                nc.vector.memset(t, 0)
                # idx value for dst (p, c) = 16*p + 4*pair + c
                it = nc.gpsimd.iota(
                    t[0:16].rearrange("p (hi lo) -> p hi lo", lo=8),
                    pattern=[[1, 4], [256, 8]],
                    base=4 * p,
                    channel_multiplier=16,
                )
                iota_insts.append(it)
                d_idx[p] = t

        # warmup matmul chain: keeps PE busy (gapless into the first real
        # matmuls) so the p-state ramp is mature when they arrive
        warm_ps = psum.tile([128, 1024] if psplit else [128, 2048], f32, name="ps")
        for _ in range(warm_n):
            nc.tensor.matmul(
                out=warm_ps[0:1, 0:256],
                lhsT=wz[:, 0:1].bitcast(f32r),
                rhs=wz[:, 0:256].bitcast(f32r),
                start=True,
                stop=True,
            )

        # --- tiles ---
        d_tiles = [dpool.tile([128, 4, 512], f32, name="d") for _ in range(NPAIR)]
        fi_tiles = [fipool.tile([128, 4, 1024], f32, name="fi") for _ in range(NBLK)]

        # --- head loads: d0 halves + fi0 halves split SP/ACT, issued first ---
        d0 = d_tiles[0].rearrange("p c f -> p (c f)")
        nc.sync.dma_start(out=d0[:, 0:1024], in_=d_t[:, 0:64, :].rearrange("v e s -> v (e s)"))
        nc.scalar.dma_start(out=d0[:, 1024:2048], in_=d_t[:, 64:128, :].rearrange("v e s -> v (e s)"))
        fb0 = fi_tiles[0].rearrange("p j f -> p (j f)")
        src0 = fi_t[:, 0:EB, :].rearrange("v e n -> v (e n)")
        nc.sync.dma_start(out=fb0[:, 0:2048], in_=src0[:, 0:2048])
        nc.scalar.dma_start(out=fb0[:, 2048:4096], in_=src0[:, 2048:4096])

        # pair-0 w pipeline in half-pair units so block 0 can start early
        with tc.high_priority():
            for h in range(2):
                sl = d_tiles[0][:, 2 * h : 2 * h + 2, :]
                nc.vector.tensor_mul(sl, sl, sl)
                nc.scalar.activation(
                    sl, sl, mybir.ActivationFunctionType.Exp, scale=-1.0
                )

        # fi1 split SP/ACT; fi7 full on SP
        fb1 = fi_tiles[1].rearrange("p j f -> p (j f)")
        src1 = fi_t[:, EB : 2 * EB, :].rearrange("v e n -> v (e n)")
        nc.sync.dma_start(out=fb1[:, 0:2048], in_=src1[:, 0:2048])
        nc.scalar.dma_start(out=fb1[:, 2048:4096], in_=src1[:, 2048:4096])
        src7 = fi_t[:, 7 * EB : 8 * EB, :].rearrange("v e n -> v (e n)")
        nc.sync.dma_start(out=fi_tiles[7].rearrange("p j f -> p (j f)"), in_=src7)
        if d_direct:
            for ps_ in d_direct.split(","):
                pp = int(ps_)
                nc.sync.dma_start(
                    out=d_tiles[pp].rearrange("p c f -> p (c f)"),
                    in_=d_t[:, 128 * pp : 128 * (pp + 1), :].rearrange("v e s -> v (e s)"),
                )
        direct_blocks = set()
        if fi_direct:
            for item in fi_direct.split(","):
                bb, mode = int(item[:-1]), item[-1]
                direct_blocks.add(bb)
                fbx = fi_tiles[bb].rearrange("p j f -> p (j f)")
                srcx = fi_t[:, bb * EB : (bb + 1) * EB, :].rearrange("v e n -> v (e n)")
                if mode == "h":
                    nc.sync.dma_start(out=fbx[:, 0:2048], in_=srcx[:, 0:2048])
                    nc.scalar.dma_start(out=fbx[:, 2048:4096], in_=srcx[:, 2048:4096])
                elif mode == "s":
                    nc.sync.dma_start(out=fbx, in_=srcx)
                elif mode == "a":
                    nc.scalar.dma_start(out=fbx, in_=srcx)

        # --- Pool gather chain: d1, fi2, d2, fi3, fi4, d3, fi5, fi6 ---
        libload = nc.gpsimd.load_library(library_config.mlp)
        deps = bass.InstructionNameOrderedSet()
        for it in iota_insts:
            deps.add(it.ins.name)
        libload.ins.add_nosync_dependencies_from(deps)
        libdep = bass.InstructionNameOrderedSet()
        libdep.add(libload.ins.name)

        chain_dep = [libload.ins.name]

        def gather_d(p):
            gi = nc.gpsimd.dma_gather(
                d_tiles[p], d_rows, d_idx[p], 512, 512, 512,
                prepare_only=True, sem=d_sem[p],
            )
            dd = bass.InstructionNameOrderedSet()
            for nm in chain_dep:
                dd.add(nm)
            gi.ins.add_nosync_dependencies_from(dd)
            chain_dep[0] = gi.ins.name
            nc.gpsimd.trigger_dma()

        def gather_fi(b):
            gi = nc.gpsimd.dma_gather(
                fi_tiles[b], fi_rows, fi_idx[b], 512, 512, 1024,
                prepare_only=True, sem=fi_sem[b],
            )
            dd = bass.InstructionNameOrderedSet()
            for nm in chain_dep:
                dd.add(nm)
            gi.ins.add_nosync_dependencies_from(dd)
            chain_dep[0] = gi.ins.name
            nc.gpsimd.trigger_dma()

        for item in chain.split(","):
            if item[0] == "d":
                gather_d(int(item[1:]))
            else:
                gather_fi(int(item[1:]))

        # --- main pipeline ---
        emap = {"s": nc.sync, "a": nc.scalar, "g": nc.gpsimd}
        store_eng = [emap[c] for c in store_map]
        for p in range(NPAIR):
            dt_ = d_tiles[p]
            direct_d = d_direct and str(p) in d_direct.split(",")
            if p > 0:
                if not direct_d:
                    if sq_act[p] == "1":
                        nc.scalar.wait_ge(d_sem[p], 16)
                    else:
                        nc.vector.wait_ge(d_sem[p], 16)
                if sq_act[p] == "1":
                    nc.scalar.activation(
                        dt_, dt_, mybir.ActivationFunctionType.Square
                    )
                else:
                    nc.vector.tensor_mul(dt_, dt_, dt_)
                nc.scalar.activation(
                    dt_, dt_, mybir.ActivationFunctionType.Exp, scale=-1.0
                )
            if psplit:
                pts = [[psum.tile([128, 1024], f32, name="ps") for _ in range(2)]
                       for _ in range(2)]
                wq = pts[0][0][:, 0:16]
            else:
                pts = [psum.tile([128, 2048], f32, name="ps") for _ in range(2)]
                wq = pts[0][:, 0:16]
            # wbar columns: 16 stacks of 8 examples -> [128, 16] psum corner
            for sig in range(16):  # sig = 8*half + 4*u + m ; pair offset 8*sig
                nc.tensor.matmul(
                    out=wq[:, sig : sig + 1],
                    lhsT=dt_[:, sig // 4, 128 * (sig % 4) : 128 * (sig % 4) + 128].bitcast(f32r),
                    rhs=ones.bitcast(f32r),
                    start=(sig == 0),
                    stop=(sig == 15),
                )
            wcol = wpool.tile([128, 16], f32, name="wcol")
            nc.vector.tensor_copy(wcol, wq)

            for half in range(2):
                b = 2 * p + half
                if psplit:
                    ptu = pts[half]
                else:
                    pt = pts[half]
                    ptv = pt.rearrange("p (u m n) -> p u m n", u=2, m=4)
                if 2 <= b <= 6 and b not in direct_blocks:
                    nc.tensor.wait_ge(fi_sem[b], 16)
                ft = fi_tiles[b]
                ot = opool.tile([128, 2, 4, 256], bf16, name="o")
                for u in range(2):
                    for gh in range(2):
                        for m in range(4):
                            E = 64 * half + 32 * u + 8 * m + 4 * gh  # pair offset
                            eb = 32 * u + 8 * m + 4 * gh             # block offset
                            mm_out = (
                                ptu[u][64 * gh : 64 * gh + 64, 256 * m : 256 * m + 256]
                                if psplit
                                else pt[
                                    64 * gh : 64 * gh + 64,
                                    1024 * u + 256 * m : 1024 * u + 256 * m + 256,
                                ]
                            )
                            nc.tensor.matmul(
                                out=mm_out,
                                lhsT=dt_[:, E // 32, 16 * (E % 32) : 16 * (E % 32) + 64].bitcast(f32r),
                                rhs=ft[:, eb // 16, 64 * (eb % 16) : 64 * (eb % 16) + 256].bitcast(f32r),
                                start=(m % 2 == 0),
                                stop=(m % 2 == 1),
                                tile_position=(0, 64 * gh),
                            )
                    # drain half-tile: fold by wcol + convert to bf16
                    use_act = (drain_act and str(b) in drain_act.split(","))
                    if drain_mode == "usplit":
                        use_act = (u == 1) and (drain_act == "" or str(b) in drain_act.split(","))
                    pin = (ptu[u].rearrange("p (m n) -> p m n", m=4)
                           if psplit else ptv[:, u])
                    if use_act:
                        for m in range(4):
                            sig = 8 * half + 4 * u + m
                            nc.scalar.activation(
                                ot[:, u, m], pin[:, m],
                                mybir.ActivationFunctionType.Copy,
                                scale=wcol[:, sig : sig + 1],
                            )
                    else:
                        wslice = wcol[:, 8 * half + 4 * u : 8 * half + 4 * u + 4]
                        wb = wslice.rearrange("p (c o) -> p c o", o=1).broadcast_to([128, 4, 256])
                        nc.vector.tensor_mul(ot[:, u], pin, wb)
                eng = store_eng[b]
                otf = ot.rearrange("p u m n -> p (u m n)")
                if b == NBLK - 1:
                    ov = out[b].rearrange("q (u f) -> q u f", u=2)
                    eng.dma_start(out=ov[:, 0], in_=otf[:, 0:1024])
                    eng.dma_start(out=ov[:, 1], in_=otf[:, 1024:2048])
                else:
                    eng.dma_start(out=out[b], in_=otf)
    return nc


_NC_CACHE = {}


def _get_nc():
    if "nc" not in _NC_CACHE:
        _NC_CACHE["nc"] = build()
    return _NC_CACHE["nc"]


def _host_pre(fi_v, d_av, core):
    sl = slice(core * BPC, (core + 1) * BPC)
    return {
        "fi_t": np.ascontiguousarray(fi_v[sl].transpose(1, 0, 2)),
        "d_t": np.ascontiguousarray(d_av[sl].transpose(1, 0, 2)),
    }


def _host_post(scratch):
    # scratch [NBLK, 128, 2048] bf16: store t covers examples 64t..64t+64
    # partition = (gh:2, k:4, s:16); col = (u:2, m:4, k2:4, n:64); valid k==k2
    # e = 64t + 32u + 8m + 4gh + k
    S_ = np.asarray(scratch, dtype=np.float32).reshape(NBLK, 2, 4, 16, 2, 4, 4, 64)
    k = np.arange(4)
    D = S_[:, :, k, :, :, :, k, :]  # [k, t, gh, s, u, m, n]
    D = D.transpose(1, 4, 5, 2, 0, 3, 6)  # [t, u, m, gh, k, s, n]
    return np.ascontiguousarray(D.reshape(BPC, S * N))


def kernel(fi_v: np.ndarray, d_av: np.ndarray) -> np.ndarray:
    from concourse.bass_utils import run_bass_kernel_spmd

    fi_v = np.ascontiguousarray(np.asarray(fi_v, dtype=np.float32))
    d_av = np.ascontiguousarray(np.asarray(d_av, dtype=np.float32))
    assert fi_v.shape == (B, V, N) and d_av.shape == (B, V, S)
    nc = _get_nc()
    in_maps = [_host_pre(fi_v, d_av, c) for c in range(NCORES)]
    res = run_bass_kernel_spmd(nc, in_maps, core_ids=list(range(NCORES)))
    return np.concatenate(
        [_host_post(res.results[c]["out"]) for c in range(NCORES)], axis=0
    )
